# revision 17
# baseline (speedup 1.0000x reference)
"""Trainium2 Bass kernel for the non-local (self-attention over spatial
positions) block.

Per batch b (8 batches -> one per NeuronCore):
    xf    = x[b]                       [C=128, N=4096]
    theta = w_theta @ xf               [64, N]
    phi   = w_phi   @ xf               [64, N]
    g     = w_g     @ xf               [64, N]
    attn  = softmax(theta^T phi)       [N, N]   (softmax over keys m)
    y     = g @ attn^T                 [64, N]
    out   = w_last @ y + xf            [128, N]

Design (per core), v4 (vs 159 us v1 baseline):
 - scoresT[m, q] orientation (phi tiles stationary) so exp(scoresT)
   feeds the y matmul directly as the moving operand.
 - fp16 theta/phi: the two concurrent 64-row score matmuls (m-tile
   pair in disjoint PE row groups) stream from separate XBUSes; fp16's
   10-bit mantissa keeps logit error ~10x below bf16's.
 - exp (16.8M elements) split 18/14 between ACT (real Exp) and DVE
   (Schraudolph bit-trick exp: one tensor_scalar affine with int16
   output bitcast as bf16), interleaved so neither engine backlogs.
 - y accumulation split h0/h64 into the two banks of one [65,1024]
   PSUM tile: the per-m-tile matmul pair runs concurrently in disjoint
   PE row groups and drains to different banks (same-bank accumulate
   serializes on the drain port at ~242ns/MM; split ~212ns per m-tile
   for both MMs).  The halves are merged for free by the two w_last
   matmuls (PSUM accumulate) and a [128,4] add for the row sums.
 - Single merged [65,1024] f32->bf16 PSUM evacuation per chunk
   (y halves + both rowsum rows) instead of separate yu/rs copies.
 - Rowsum reciprocal via two DMA scatters [1,512]->[128,4], add,
   [128,4] reciprocal, DMA gather back (a [1,512] DVE reciprocal
   costs 3.3us and starves the DVE exp stream) -- used for the final
   chunk too (scatter chain ~1.5us latency vs 3.3us direct).
 - GPSIMD runs only off-critical-path ops (partition broadcast,
   residual add): its copies cost ~1.9us and its semaphore ops ~770ns.
 - 8 warmup matmuls on a memset tile at t=0 push HAM toward K=8/8
   (2.4 GHz) before the fused init+chunk-0 phase.
 - Input DMA / projections / first q-chunk fused so the PE starts
   while the input streams in; weights DMA'd before the bulk input.
"""

import sys

import numpy as np

for _p in ("/opt/trn_rl_repo",):
    if _p not in sys.path:
        sys.path.insert(0, _p)

import concourse.bass as bass
from concourse import bacc
import concourse.mybir as mybir
import concourse.tile as tile
from concourse.alu_op_type import AluOpType
from concourse.bass_utils import run_bass_kernel_spmd

F32 = mybir.dt.float32
F16 = mybir.dt.float16
BF16 = mybir.dt.bfloat16
I16 = mybir.dt.int16

P = 128     # channels C / partition dim
CB = 64     # bottleneck channels
NQ = 4096   # spatial positions (64*64)
NMT = 32    # m (key) tiles of 128
NPAIR = 16  # m-tile pairs per q chunk
NQC = 8     # q chunks of 512

# Schraudolph exp in bf16: exp(s) ~= bitcast<bf16>(int16(A*s + B))
EXP_A = float(2**7 / np.log(2.0))
EXP_B = float(127.0 * 2**7 - 6.0)
# exp engine split: DVE takes the second tile of every pair but the
# first (15 of 32 tiles); ACT takes the other 17.
def _exp_on_dve(mi):
    return (mi % 2 == 1) and (mi != 1)

_NC_CACHE = {}


def _build():
    nc = bacc.Bacc()
    x_in = nc.declare_dram_parameter("xb16", [P, NQ], F16, isOutput=False)
    wqk_in = nc.declare_dram_parameter("wqk", [P, P], F16, isOutput=False)
    wg_in = nc.declare_dram_parameter("wgT", [P, CB], F16, isOutput=False)
    wl_in = nc.declare_dram_parameter("wl", [CB, P], BF16, isOutput=False)
    out_d = nc.declare_dram_parameter("out", [P, NQ], F32, isOutput=True)

    with tile.TileContext(nc) as tc:
        with (
            tc.tile_pool(name="const", bufs=1) as const,
            tc.tile_pool(name="big", bufs=1) as big,
            tc.tile_pool(name="work", bufs=2) as work,
            tc.tile_pool(name="probs", bufs=9) as probs,
            tc.tile_pool(name="spool", bufs=6, space="PSUM") as spool,
            tc.tile_pool(name="ypool", bufs=1, space="PSUM") as ypool,
        ):
            # ---- small weights first (needed before any compute);
            # all operand casts were done host-side ----
            wqk = const.tile([P, P], F16)
            wg = const.tile([P, CB], F16)
            wl = const.tile([CB, P], BF16)
            nc.sync.dma_start(out=wqk, in_=wqk_in[:, :])
            nc.sync.dma_start(out=wg, in_=wg_in[:, :])
            nc.sync.dma_start(out=wl, in_=wl_in[:, :])

            xb16 = big.tile([P, NQ], F16)
            # theta/phi duplicated into both partition halves so score
            # matmuls for two m-tiles run concurrently in PE row groups
            theta = big.tile([P, NQ], F16)
            phi = big.tile([P, NQ], F16)
            # gT in 65-col slots; col 64 = ones for the row-sum trick
            gt = big.tile([P, NMT * (CB + 1)], BF16)
            nc.vector.memset(gt, 1.0)
            gt3 = gt.rearrange("p (m c) -> p m c", c=CB + 1)

            # ---------------- pipelined helpers ----------------
            qof = [qc * 512 for qc in range(NQC)]

            def score_pair(qc, pi):
                """Two concurrent 64-row score matmuls for m-tiles
                2*pi (rows 0:64) and 2*pi+1 (rows 64:128)."""
                q = qof[qc]
                sa = spool.tile([P, 512], F32, tag="s")
                nc.tensor.matmul(
                    sa, phi[0:CB, (2 * pi) * 128:(2 * pi + 1) * 128],
                    theta[0:CB, q:q + 512], start=True, stop=True,
                )
                sb = spool.tile([P, 512], F32, tag="s")
                nc.tensor.matmul(
                    sb, phi[CB:P, (2 * pi + 1) * 128:(2 * pi + 2) * 128],
                    theta[CB:P, q:q + 512], start=True, stop=True,
                )
                return sa, sb

            def exp_tile(mi, sp, split=False):
                pb = probs.tile([P, 512], BF16, tag="pb")
                if split:
                    # tail of the last chunk: halve drain latency by
                    # computing one half on each engine concurrently
                    nc.scalar.activation(
                        pb[:, 0:256], sp[:, 0:256],
                        mybir.ActivationFunctionType.Exp,
                    )
                    nc.vector.tensor_scalar(
                        pb.bitcast(I16)[:, 256:512], sp[:, 256:512],
                        EXP_A, EXP_B, AluOpType.mult, AluOpType.add,
                    )
                elif _exp_on_dve(mi):
                    nc.vector.tensor_scalar(
                        pb.bitcast(I16), sp, EXP_A, EXP_B,
                        AluOpType.mult, AluOpType.add,
                    )
                else:
                    nc.scalar.activation(
                        pb, sp, mybir.ActivationFunctionType.Exp
                    )
                return pb

            def y_mm(yps, mi, pb):
                """h0/h64 contraction halves -> the two banks of yps;
                merged later by the two w_last matmuls + rowsum add."""
                slot = slice(mi * (CB + 1), (mi + 1) * (CB + 1))
                st = (mi == 0)
                sp_ = (mi == NMT - 1)
                nc.tensor.matmul(
                    yps[:, 0:512], gt[0:CB, slot], pb[0:CB, :],
                    start=st, stop=sp_,
                )
                nc.tensor.matmul(
                    yps[:, 512:1024], gt[CB:P, slot], pb[CB:P, :],
                    start=st, stop=sp_,
                )

            # per-chunk epilogue, staged across the next chunk's pair
            # stream so no single engine stalls the PE.
            #   chunk end: merged ym copy (DVE, frees yps)
            #   pi==2: DMA scatter both rowsum rows -> [128,4] each
            #   pi==3: add halves -> [128,4] f32 (DVE)
            #   pi==4: reciprocal [128,4] (DVE)
            #   pi==5: DMA gather -> [1, 512]
            #   pi==6: partition broadcast (gpsimd)
            #   pi==7: two w_last matmuls, PSUM-accumulated merge (PE)
            #   pi==8: normalize multiply (DVE, straight out of PSUM)
            #   pi==9: residual add (gpsimd)
            #   pi==10: output DMA
            def epi_stage(st, stage):
                qc = st["qc"]
                if stage == 2:
                    st["rsa"] = work.tile([P, 4], BF16, tag="rsa", name="rsa")
                    st["rsb"] = work.tile([P, 4], BF16, tag="rsb", name="rsb")
                    nc.sync.dma_start(out=st["rsa"], in_=st["ym"][CB:CB + 1, 0:512])
                    nc.sync.dma_start(out=st["rsb"], in_=st["ym"][CB:CB + 1, 512:1024])
                elif stage == 3:
                    st["rq4"] = work.tile([P, 4], F32, tag="rq4", name="rq4")
                    nc.vector.tensor_add(st["rq4"], st["rsa"], st["rsb"])
                elif stage == 4:
                    st["rqi"] = work.tile([P, 4], F32, tag="rqi", name="rqi")
                    nc.vector.reciprocal(st["rqi"], st["rq4"])
                elif stage == 5:
                    st["rinv"] = work.tile([1, 512], F32, tag="rinv", name="rinv")
                    nc.sync.dma_start(out=st["rinv"], in_=st["rqi"])
                elif stage == 6:
                    st["rb"] = work.tile([P, 512], F32, tag="rb", name="rb", bufs=3)
                    nc.gpsimd.partition_broadcast(st["rb"], st["rinv"])
                elif stage == 7:
                    st["op"] = spool.tile([P, 512], F32, tag="s", name="op")
                    nc.tensor.matmul(st["op"], wl, st["ym"][0:CB, 0:512],
                                     start=True, stop=False)
                    nc.tensor.matmul(st["op"], wl, st["ym"][0:CB, 512:1024],
                                     start=False, stop=True)
                elif stage == 8:
                    st["ob"] = work.tile([P, 512], F32, tag="ob", name="ob", bufs=3)
                    nc.vector.tensor_mul(st["ob"], st["op"], st["rb"])
                elif stage == 9:
                    # residual add on DVE: the out-DMA waits on this, and
                    # a slow producer (gpsimd) stalls the whole DMA queue
                    # (head-of-line) delaying the NEXT chunk's scatters
                    st["ob2"] = work.tile([P, 512], F32, tag="ob2", name="ob2", bufs=3)
                    nc.vector.tensor_add(
                        st["ob2"], st["ob"], xb16[:, qof[qc]:qof[qc] + 512]
                    )
                elif stage == 10:
                    nc.sync.dma_start(
                        out=out_d[:, qof[qc]:qof[qc] + 512], in_=st["ob2"]
                    )

            EPI_STAGES = range(2, 11)

            def epi_begin(qc, yps):
                # merged evacuation, one bank-half per engine (ACT and
                # DVE may touch PSUM concurrently on different banks);
                # subtile deps let the next chunk's y matmuls start as
                # soon as their half is freed
                ym = work.tile([CB + 1, 1024], BF16, tag="ym", name="ym")
                nc.scalar.copy(ym[:, 0:512], yps[:, 0:512])
                nc.vector.tensor_copy(ym[:, 512:1024], yps[:, 512:1024])
                return {"qc": qc, "ym": ym}

            def pair_tiles(qc, yps, pi, look=2):
                """Scores+exp for pair pi, y matmuls for pair pi-look."""
                sa, sb = score_pair(qc, pi)
                split = (qc == NQC - 1 and pi >= NPAIR - 3)
                pbq[2 * pi] = exp_tile(2 * pi, sa, split)
                pbq[2 * pi + 1] = exp_tile(2 * pi + 1, sb, split)
                pj = pi - look
                if pj >= 0:
                    for mi in (2 * pj, 2 * pj + 1):
                        y_mm(yps, mi, pbq.pop(mi))

            def drain_y(yps, look=2):
                for pj in range(NPAIR - look, NPAIR):
                    for mi in (2 * pj, 2 * pj + 1):
                        y_mm(yps, mi, pbq.pop(mi))

            # ---------------- init: projections only ----------------
            # Per 512-col xb chunk j: DMA, theta/phi projection, 4 gT
            # projections.  Attention runs as 8 uniform chunks after;
            # fusing chunk-0 into this loop just interleaves work on
            # the same two evac engines and adds dependency stalls.
            pbq = {}  # mi -> pb tile awaiting its y matmul

            for j in range(8):
                cs = slice(j * 512, (j + 1) * 512)
                nc.sync.dma_start(out=xb16[:, cs], in_=x_in[:, cs])
                ps = spool.tile([P, 512], F32, tag="s")
                nc.tensor.matmul(ps, wqk, xb16[:, cs], start=True, stop=True)
                # theta lower half is partition-aligned -> ACT engine;
                # the shifted copies (DVE only) fill the other halves
                nc.scalar.copy(theta[0:CB, cs], ps[0:CB, :])
                nc.vector.tensor_copy(phi[0:CB, cs], ps[CB:P, :])
                nc.vector.tensor_copy(phi[CB:P, cs], ps[CB:P, :])
                if j == 0:
                    nc.vector.tensor_copy(theta[CB:P, cs], ps[0:CB, :])
                gp = spool.tile([P, 512], F32, tag="s")
                gp3 = gp.rearrange("p (m c) -> p m c", c=CB)
                for k in range(4):
                    mi = 4 * j + k
                    nc.tensor.matmul(
                        gp3[:, k, :], xb16[:, mi * 128:(mi + 1) * 128], wg,
                        start=True, stop=True,
                    )
                nc.scalar.copy(gt3[:, 4 * j:4 * j + 4, 0:CB], gp3[:, 0:4, :])

            # ---- PE warmup right before the attention stream: HAM
            # releases the clock gate (1.2->2.4GHz) only after ~3.4us
            # of sustained PE activity, and the sparse projection phase
            # never flips it.  Dummy matmuls into the (still free)
            # ypool banks; chunk-0's first y matmul clears them.
            warmy = ypool.tile([CB + 1, 1024], F32, tag="y", name="warmy")
            for w in range(12):
                nc.tensor.matmul(
                    warmy[:, (w % 2) * 512:(w % 2) * 512 + 512],
                    xb16[0:CB, 0:CB + 1], xb16[0:CB, 0:512],
                    start=True, stop=True,
                )

            # ---------------- q-chunks 0..7 ----------------
            pending = None
            for qc in range(NQC):
                yps = ypool.tile([CB + 1, 1024], F32, tag="y")
                for pi in range(NPAIR):
                    pair_tiles(qc, yps, pi)
                    if pending is not None and pi in EPI_STAGES:
                        epi_stage(pending, pi)
                    if pi == 11 and qc < NQC - 1:
                        # upper-theta for the next chunk: SBUF-only, so
                        # gpsimd (idle) takes it off the evac engines
                        cn = slice(qof[qc + 1], qof[qc + 1] + 512)
                        nc.gpsimd.tensor_copy(theta[CB:P, cn], theta[0:CB, cn])
                drain_y(yps)
                pending = epi_begin(qc, yps)

            # final epilogue: w_last matmuls first (need only ym), the
            # reciprocal DMA chain in parallel on sync/DVE/gpsimd
            st = pending
            for stage in (7, 2, 3, 4, 5, 6, 8, 9, 10):
                epi_stage(st, stage)

    nc.finalize()
    return nc


def kernel(x, w_theta, w_phi, w_g, w_last):
    import ml_dtypes

    B, C, H, W = x.shape
    N = H * W
    xf = np.ascontiguousarray(
        x.reshape(B, C, N).astype(np.float16)
    )
    wqk = np.ascontiguousarray(
        np.concatenate([w_theta.T, w_phi.T], axis=1), dtype=np.float16
    )
    wgT = np.ascontiguousarray(w_g.T, dtype=np.float16)
    wl = np.ascontiguousarray(w_last.T.astype(ml_dtypes.bfloat16))

    if "nc" not in _NC_CACHE:
        _NC_CACHE["nc"] = _build()
    nc = _NC_CACHE["nc"]

    in_maps = [
        {"xb16": xf[b], "wqk": wqk, "wgT": wgT, "wl": wl} for b in range(B)
    ]
    r = run_bass_kernel_spmd(nc, in_maps, list(range(B)))
    out = np.stack([r.results[b]["out"] for b in range(B)], axis=0)
    return out.reshape(B, C, H, W).astype(np.float32)
